# revision 45
# baseline (speedup 1.0000x reference)
"""GPT2-style fused attention (DecisionTransformer) on 8 Trainium2 NeuronCores.

Sharding: batch x head-group.  Core c = b*4 + g handles batch b and heads
4g..4g+3.  The host pre-transposes each batch's activations to x^T [D, S]
and casts to bf16 (layout prep during sharding), so the QKV projection
directly produces Q^T/K^T/V^T feature-major tiles -- no on-chip input
transposes at all.

Per core, fully software-pipelined:
  - QKV runs per 512-token chunk (u0..u3); u0 is a standalone prologue
    (DMA-gated), later chunks are interleaved as PE "fillers" into the
    Act-engine-gated attention rounds so the PE never idles.
  - V returns to token-major layout via the DMA transpose engine (xbar),
    augmented with 64 ones-columns FIRST: the A@V matmul then yields the
    softmax denominator replicated on PSUM partitions 0:64 (base 0, where
    the custom-DVE reciprocal works) and O^T on partitions 64:128.
  - causal attention per head: scores^T = K^T-block @ Q^T (Q zero-padded
    to 128 contraction rows), exp without max-subtraction (logits are
    small and bounded).  The (head, key-block) stream runs scores one
    unit ahead of the A@V consumers, ACROSS head boundaries, so neither
    the PE nor the Activation engine ever drains.
  - row-parallel output projection with its 256 rows of c_proj_w,
    interleaved one query-block per head into the NEXT attention round,
    written as a full-shape bf16 partial [2048, 1024].
Host sums the 4 partials per batch (row-parallel all-reduce) + bias.

Precision: Q/K projections run in fp8e4m3 with the DoubleRow perf mode
(two 128-row contraction tiles per pass, 2x PE rate); V projection and
everything downstream run in bf16 (an fp8 V feeds the output linearly and
alone pushes the error to ~4e-2).  Measured end-to-end error 1.45e-2
relative to the fp32 reference's absmax, vs the 2e-2 gate.
"""

import sys

for _p in ("/opt/trn_rl_repo",):
    if _p not in sys.path:
        sys.path.insert(0, _p)

import numpy as np
import ml_dtypes

import concourse.bass as bass
import concourse.mybir as mybir
import concourse.tile as tile
from concourse import bacc
from concourse.bass_utils import run_bass_kernel_spmd

P = 128
B, S, D, H, HD = 2, 2048, 1024, 16, 64
KO = D // P            # 8 contraction blocks
NH = 4                 # heads per core
NF = 6                 # feature blocks: q01 q23 k01 k23 v01 v23
QC = 512               # query chunk
NQC = S // QC          # 4
NKB = S // P           # 16 key blocks
SCALE = 1.0 / float(HD) ** 0.5
N_CORES = 8
N_WARM = 36
# Q/K run in fp8e4m3 with DoubleRow (2 contraction-tiles per pass).
# c_attn_w's Q/K columns are pre-scaled by 64 on the host so the
# ~N(0, 0.02) entries sit in fp8's normal range; Q/K come out x64 and the
# x4096 on the scores is folded into the exp scale.  V stays bf16 at true
# scale, so w_proj needs no compensation.
WSCALE = 64.0
EXP_SCALE = SCALE / (WSCALE * WSCALE)

f32 = mybir.dt.float32
bf16 = mybir.dt.bfloat16
fp8 = mybir.dt.float8e4
EXP = mybir.ActivationFunctionType.Exp
ADD = mybir.AluOpType.add
MULT = mybir.AluOpType.mult


def _build_program(debug=False):
    nc = bacc.Bacc(None, target_bir_lowering=False)

    x_d = nc.dram_tensor("x_t", [D, S], fp8, kind="ExternalInput")
    xb_d = nc.dram_tensor("x_tb", [D, S], bf16, kind="ExternalInput")
    wq_d = nc.dram_tensor("w_qkv", [D, 4 * P], fp8, kind="ExternalInput")
    wv_d = nc.dram_tensor("w_v", [D, 2 * P], bf16, kind="ExternalInput")
    bq_d = nc.dram_tensor("b_qkv", [NF * P], f32, kind="ExternalInput")
    wp_d = nc.dram_tensor("w_proj", [2 * P, D], bf16, kind="ExternalInput")
    out_d = nc.dram_tensor("out", [S, D], bf16, kind="ExternalOutput")
    if debug:
        dbg_d = {
            nm: nc.dram_tensor(nm, [P, S], bf16, kind="ExternalOutput")
            for nm in ("d_qpad0", "d_qpad1", "d_kt0", "d_vt0", "d_vaug0", "d_xt0")
        }
        dbg_d["d_po00"] = nc.dram_tensor("d_po00", [P, QC], f32, kind="ExternalOutput")
        dbg_d["d_rbs00"] = nc.dram_tensor("d_rbs00", [HD, QC], f32, kind="ExternalOutput")
        dbg_d["d_atn0"] = nc.dram_tensor("d_atn0", [P, QC], bf16, kind="ExternalOutput")

    with tile.TileContext(nc) as tc:
        with (
            tc.tile_pool(name="const", bufs=1) as const,
            tc.tile_pool(name="pt", bufs=6) as pt_pool,
            tc.tile_pool(name="atn", bufs=2) as atn_pool,
            tc.tile_pool(name="outp", bufs=4) as out_pool,
            tc.tile_pool(name="small", bufs=3) as small_pool,
            tc.tile_pool(name="ps_mm", bufs=2, space="PSUM") as ps_mm,
            tc.tile_pool(name="ps_s", bufs=4, space="PSUM") as ps_s,
            tc.tile_pool(name="ps_o", bufs=2, space="PSUM") as ps_o,
        ):
            # ---- input DMAs, priority order: what the first QKV chunk
            # needs arrives first; later x^T chunks and proj weights are
            # issued after the u0 compute is emitted so they don't steal
            # queue bandwidth from the critical u0 transfer.
            wq_sb = const.tile([P, KO, 4 * P], fp8)
            wqr = wq_d.rearrange("(ko p) f -> p ko f", p=P)
            wv_sb = const.tile([P, KO, 2 * P], bf16)
            wvr = wv_d.rearrange("(ko p) f -> p ko f", p=P)
            xt = const.tile([P, KO, S], fp8)
            xr = x_d.rearrange("(ko p) t -> p ko t", p=P)
            xtb = const.tile([P, KO, S], bf16)
            xbr = xb_d.rearrange("(ko p) t -> p ko t", p=P)
            # V path (bf16, 2x the bytes) first: it gates V_aug and attn0
            nc.sync.dma_start(wv_sb[:], wvr[:])
            nc.sync.dma_start(xtb[:, :, :QC], xbr[:, :, :QC])
            nc.sync.dma_start(wq_sb[:], wqr[:])
            nc.sync.dma_start(xt[:, :, :QC], xr[:, :, :QC])
            bq_sb = const.tile([P, NF], f32)
            nc.sync.dma_start(bq_sb[:], bq_d.rearrange("(c p) -> p c", p=P))
            nc.sync.dma_start(xtb[:, :, QC : 2 * QC], xbr[:, :, QC : 2 * QC])
            nc.sync.dma_start(xt[:, :, QC : 2 * QC], xr[:, :, QC : 2 * QC])
            wp_sb = const.tile([P, 2, D], bf16)

            # ---- constants ----
            # mask[k, q] = 1.0 if k <= q else 0.0 (diagonal 128-blocks)
            mask_f = const.tile([P, P], f32)
            nc.gpsimd.memset(mask_f[:], 1.0)
            nc.gpsimd.affine_select(
                out=mask_f[:], in_=mask_f[:],
                compare_op=mybir.AluOpType.is_ge, fill=0.0,
                base=0, pattern=[[1, P]], channel_multiplier=-1,
            )
            mask = const.tile([P, P], bf16)
            nc.vector.tensor_copy(mask[:], mask_f[:])

            # PE p-state warmup during the input DMAs
            warm = const.tile([P, P], bf16)
            nc.gpsimd.memset(warm[:], 0.25)
            for _ in range(N_WARM):
                psw = ps_s.tile([P, QC], f32, tag="s", name="psw")
                nc.tensor.matmul(psw[:, :P], warm[:], warm[:], start=True, stop=True)

            # ---- persistent QKV^T tiles ----
            # Q^T per head, zero-padded to 128 contraction rows (even heads
            # live in rows 0:64, odd in 64:128 -- matching the stacked K^T
            # pair tiles, so a full-128-partition matmul contracts exactly).
            qpad = [const.tile([P, S], bf16, tag=f"qp{h}", name=f"qp{h}")
                    for h in range(NH)]
            kt = [const.tile([P, S], bf16, tag=f"kt{r}", name=f"kt{r}")
                  for r in range(2)]
            vt = [const.tile([P, S], bf16, tag=f"vt{r}", name=f"vt{r}")
                  for r in range(2)]
            # V_aug[token, 0:64] = 1.0 (denominator columns, base 0 for the
            # custom-DVE reciprocal); [token, 64:128] = V features.
            vaug = [const.tile([P, NKB, P], bf16, tag=f"va{h}", name=f"va{h}")
                    for h in range(NH)]
            for h in range(NH):
                pad = qpad[h][HD:, :] if h % 2 == 0 else qpad[h][:HD, :]
                nc.gpsimd.memset(pad, 0.0)
                nc.gpsimd.memset(vaug[h][:, :, :HD], 1.0)

            def qkv_fcs(u, fcs):
                ucs = slice(u * QC, (u + 1) * QC)
                for fc in fcs:
                    ps = ps_mm.tile([P, QC], f32, tag="mm", name="psq")
                    if fc < 4:   # Q/K: fp8 DoubleRow, 2 K-tiles per pass
                        for kp in range(KO // 2):
                            nc.tensor.matmul(
                                ps[:],
                                wq_sb[:, 2 * kp : 2 * kp + 2, fc * P : (fc + 1) * P],
                                xt[:, 2 * kp : 2 * kp + 2, ucs],
                                start=(kp == 0),
                                stop=(kp == KO // 2 - 1),
                                perf_mode=mybir.MatmulPerfMode.DoubleRow,
                            )
                    else:        # V: bf16 (fp8 V dominates output error)
                        for ko in range(KO):
                            nc.tensor.matmul(
                                ps[:],
                                wv_sb[:, ko, (fc - 4) * P : (fc - 3) * P],
                                xtb[:, ko, ucs],
                                start=(ko == 0),
                                stop=(ko == KO - 1),
                            )
                    if fc < 2:  # q01 / q23 -> per-head padded Q^T
                        h0, h1 = 2 * fc, 2 * fc + 1
                        nc.vector.tensor_scalar(
                            qpad[h0][:HD, ucs], ps[:HD],
                            bq_sb[:HD, fc : fc + 1], None, ADD)
                        nc.vector.tensor_scalar(
                            qpad[h1][HD:, ucs], ps[HD:],
                            bq_sb[HD:, fc : fc + 1], None, ADD)
                    else:
                        dst = kt[fc - 2] if fc < 4 else vt[fc - 4]
                        nc.vector.tensor_scalar(
                            dst[:, ucs], ps[:],
                            bq_sb[:, fc : fc + 1], None, ADD)

            def vaug_u(u, eng=None):
                # V^T [64 feats, 512 tokens] -> token-major via xbar DMA
                eng = eng if eng is not None else nc.sync
                for h in range(NH):
                    pr, hl = divmod(h, 2)
                    eng.dma_start_transpose(
                        vaug[h][:, u * 4 : (u + 1) * 4, HD:],
                        vt[pr][hl * HD : (hl + 1) * HD, u * QC : (u + 1) * QC],
                    )

            def proj_qb_group(qc, atns, qb, pool=None, tag="mm"):
                pool = pool if pool is not None else ps_mm
                pps = [pool.tile([P, QC], f32, tag=tag, name="pp")
                       for _ in range(2)]
                for pr in range(2):
                    for nck in range(2):
                        nc.tensor.matmul(
                            pps[nck][:],
                            atns[pr][:, qb * P : (qb + 1) * P],
                            wp_sb[:, pr, nck * QC : (nck + 1) * QC],
                            start=(pr == 0), stop=(pr == 1),
                        )
                for nck in range(2):
                    ot = out_pool.tile([P, QC], bf16, tag="ot", name="ot")
                    nc.vector.tensor_copy(ot[:], pps[nck][:])
                    row = qc * QC + qb * P
                    nc.sync.dma_start(
                        out_d[row : row + P, nck * QC : (nck + 1) * QC],
                        ot[:],
                    )

            def attn_qc(qc, atns, proj_prev=None, fillers=None, dbg=None):
                """One attention round, (head, key-block) software-pipelined:
                scores run one unit ahead of A@V across head boundaries.
                After each head's normalize: the previous round's projection
                query-block, then that head's PE filler (QKV chunk work)."""
                qcs = slice(qc * QC, (qc + 1) * QC)
                pos = [None] * NH

                def emit_scores(h, kind, kb):
                    pr = h // 2
                    ps = ps_s.tile([P, QC], f32, tag="s", name="pss")
                    pt = pt_pool.tile([P, QC], bf16, tag="pt", name="pt")
                    if kind == "o":
                        nc.tensor.matmul(
                            ps[:], kt[pr][:, kb * P : (kb + 1) * P],
                            qpad[h][:, qcs], start=True, stop=True)
                        nc.scalar.activation(pt[:], ps[:], EXP, scale=EXP_SCALE)
                    else:
                        lo = (kb - qc * 4) * P
                        nc.tensor.matmul(
                            ps[:, lo:], kt[pr][:, kb * P : (kb + 1) * P],
                            qpad[h][:, qc * QC + lo : (qc + 1) * QC],
                            start=True, stop=True)
                        nc.scalar.activation(pt[:, lo:], ps[:, lo:], EXP,
                                             scale=EXP_SCALE)
                        nc.vector.tensor_tensor(
                            pt[:, lo : lo + P], pt[:, lo : lo + P],
                            mask[:], MULT)
                    return pt

                def emit_av(h, kind, kb, pt, first, last):
                    if pos[h] is None:
                        pos[h] = ps_o.tile([P, QC], f32, tag="po", name="po")
                    po = pos[h]
                    if kind == "o":
                        nc.tensor.matmul(po[:], vaug[h][:, kb, :], pt[:],
                                         start=first, stop=last)
                    else:
                        lo = (kb - qc * 4) * P
                        nc.tensor.matmul(po[:, lo:], vaug[h][:, kb, :],
                                         pt[:, lo:], start=first, stop=last)

                def finish_head(h):
                    pr, hl = divmod(h, 2)
                    po = pos[h]
                    # denominator arrives replicated on po[0:64] (base 0)
                    rbs = small_pool.tile([HD, QC], f32, tag="rbs", name="rbs")
                    nc.vector.reciprocal_approx_fast(out=rbs[:], in_=po[:HD, :])
                    if dbg is not None and h == 0:
                        pocp = out_pool.tile([P, QC], f32, tag="dbg", name="dbg")
                        nc.vector.tensor_copy(pocp[:], po[:])
                        nc.sync.dma_start(dbg["d_po00"][:], pocp[:])
                        nc.sync.dma_start(dbg["d_rbs00"][:], rbs[:])
                    nc.vector.tensor_tensor(
                        atns[pr][hl * HD : (hl + 1) * HD, :],
                        po[HD:, :], rbs[:], MULT,
                    )
                    if proj_prev is not None:
                        proj_qb_group(proj_prev[0], proj_prev[1], qb=h)
                    if fillers is not None and fillers[h] is not None:
                        fillers[h]()

                stream = []
                for h in range(NH):
                    units = [("o", kb) for kb in range(qc * 4)] + \
                            [("d", qc * 4 + j) for j in range(4)]
                    for i, (kind, kb) in enumerate(units):
                        stream.append((h, kind, kb, i == 0,
                                       i == len(units) - 1))

                pending = None
                for h, kind, kb, first, last in stream:
                    pt = emit_scores(h, kind, kb)
                    if pending is not None:
                        ph, pk, pkb, ppt, pf, pl = pending
                        emit_av(ph, pk, pkb, ppt, pf, pl)
                        if pl:
                            finish_head(ph)
                    pending = (h, kind, kb, pt, first, last)
                ph, pk, pkb, ppt, pf, pl = pending
                emit_av(ph, pk, pkb, ppt, pf, pl)
                finish_head(ph)

            def mk_atns():
                return [atn_pool.tile([P, QC], bf16, tag=f"atn{r}",
                                      name=f"atn{r}") for r in range(2)]

            # ---- schedule ----
            # V projections first (their bf16 inputs arrive first and they
            # gate V_aug + attn0); Q/K fp8 matmuls follow.
            qkv_fcs(0, (4, 5, 0, 1, 2, 3))
            # deferred input DMAs (u2/u3/wp): issued behind u0/u1 on the
            # sync queue.  vaug_u(0)'s xbar transposes dispatch from the
            # (then-idle) Activation queue so they don't block these.
            for u in range(2, 4):
                nc.sync.dma_start(xt[:, :, u * QC : (u + 1) * QC],
                                  xr[:, :, u * QC : (u + 1) * QC])
                nc.sync.dma_start(xtb[:, :, u * QC : (u + 1) * QC],
                                  xbr[:, :, u * QC : (u + 1) * QC])
            nc.sync.dma_start(wp_sb[:], wp_d.rearrange("(c p) d -> p c d", p=P))
            vaug_u(0, eng=nc.scalar)
            a0 = mk_atns()
            attn_qc(0, a0, fillers=[
                lambda: qkv_fcs(1, (0, 1)),
                lambda: qkv_fcs(1, (2, 3)),
                lambda: qkv_fcs(1, (4, 5)),
                lambda: vaug_u(1),
            ], dbg=(dbg_d if debug else None))
            if debug:
                nc.sync.dma_start(dbg_d["d_atn0"][:], a0[0][:])
            a1 = mk_atns()
            attn_qc(1, a1, proj_prev=(0, a0), fillers=[
                lambda: qkv_fcs(2, (0, 1)),
                lambda: qkv_fcs(2, (2, 3)),
                lambda: qkv_fcs(2, (4, 5)),
                lambda: vaug_u(2),
            ])
            a2 = mk_atns()
            attn_qc(2, a2, proj_prev=(1, a1), fillers=[
                lambda: qkv_fcs(3, (0, 1)),
                lambda: qkv_fcs(3, (2, 3)),
                lambda: qkv_fcs(3, (4, 5)),
                lambda: vaug_u(3),
            ])
            a3 = mk_atns()
            attn_qc(3, a3, proj_prev=(2, a2))
            # final projection runs in the (now idle) score pool: 4-deep
            # PSUM rotation so the evict casts pipeline behind the matmuls
            for qb in range(4):
                proj_qb_group(3, a3, qb, pool=ps_s, tag="s")

            if debug:
                nc.sync.dma_start(dbg_d["d_qpad0"][:], qpad[0][:])
                nc.sync.dma_start(dbg_d["d_qpad1"][:], qpad[1][:])
                nc.sync.dma_start(dbg_d["d_kt0"][:], kt[0][:])
                nc.sync.dma_start(dbg_d["d_vt0"][:], vt[0][:])
                nc.sync.dma_start(
                    dbg_d["d_vaug0"][:],
                    vaug[0].rearrange("p a b -> p (a b)"),
                )
                nc.sync.dma_start(dbg_d["d_xt0"][:], xt[:, 0, :])

    nc.compile()
    return nc


_CACHE = {}


def get_program():
    if "p" not in _CACHE:
        _CACHE["p"] = _build_program()
    return _CACHE["p"]


def make_in_maps(hidden_states, c_attn_w, c_attn_b, c_proj_w):
    x = np.asarray(hidden_states, dtype=np.float32).reshape(B, S, D)
    wa = np.asarray(c_attn_w, dtype=np.float32)
    ba = np.asarray(c_attn_b, dtype=np.float32)
    wp = np.asarray(c_proj_w, dtype=np.float32)
    bf = ml_dtypes.bfloat16

    f8 = ml_dtypes.float8_e4m3
    xts = [np.ascontiguousarray(x[b].T).astype(f8) for b in range(B)]
    xtbs = [np.ascontiguousarray(x[b].T).astype(bf) for b in range(B)]
    in_maps = []
    for c in range(N_CORES):
        b, g = divmod(c, 4)
        w_blocks, b_blocks = [], []
        for m in range(3):          # q, k, v
            base = m * D + g * 256
            for half in range(2):   # heads (0,1) then (2,3) of the group
                w_blocks.append(wa[:, base + half * P : base + (half + 1) * P])
                b_blocks.append(ba[base + half * P : base + (half + 1) * P])
        # block order q01 q23 k01 k23 | v01 v23; Q/K x64 pre-scale for fp8
        w_qkv = np.ascontiguousarray(
            np.concatenate(w_blocks[:4], axis=1) * WSCALE).astype(f8)
        w_v = np.ascontiguousarray(
            np.concatenate(w_blocks[4:], axis=1)).astype(bf)
        b_qkv = np.ascontiguousarray(np.concatenate(
            [bb * WSCALE for bb in b_blocks[:4]] + b_blocks[4:]))
        w_proj = np.ascontiguousarray(wp[g * 256 : (g + 1) * 256, :]).astype(bf)
        in_maps.append({
            "x_t": xts[b],
            "x_tb": xtbs[b],
            "w_qkv": w_qkv,
            "w_v": w_v,
            "b_qkv": b_qkv,
            "w_proj": w_proj,
        })
    return in_maps


def kernel(hidden_states, c_attn_w, c_attn_b, c_proj_w, c_proj_b):
    nc = get_program()
    in_maps = make_in_maps(hidden_states, c_attn_w, c_attn_b, c_proj_w)
    res = run_bass_kernel_spmd(nc, in_maps, list(range(N_CORES)))
    bias = np.asarray(c_proj_b, dtype=np.float32)[None, :]
    outs = []
    for b in range(B):
        acc = res.results[b * 4]["out"].astype(np.float32)
        for g in range(1, 4):
            acc = acc + res.results[b * 4 + g]["out"].astype(np.float32)
        outs.append(acc + bias)
    return np.stack(outs).reshape(B, S, D).astype(np.float32)


if __name__ == "__main__":
    rng = np.random.default_rng(0)
    hs = rng.standard_normal((B, S, D), dtype=np.float32)
    wa = rng.standard_normal((D, 3 * D), dtype=np.float32) * 0.02
    ba = rng.standard_normal((3 * D,), dtype=np.float32) * 0.02
    wp = rng.standard_normal((D, D), dtype=np.float32) * 0.02
    bp = rng.standard_normal((D,), dtype=np.float32) * 0.02
    out = kernel(hs, wa, ba, wp, bp)
    print("out", out.shape, out.dtype, float(np.abs(out).max()))


# revision 47
# speedup vs baseline: 1.1599x; 1.1599x over previous
"""GPT2-style fused attention (DecisionTransformer) on 8 Trainium2 NeuronCores.

Sharding: batch x head-group.  Core c = b*4 + g handles batch b and heads
4g..4g+3.  The host pre-transposes each batch's activations to x^T [D, S]
and casts to bf16 (layout prep during sharding), so the QKV projection
directly produces Q^T/K^T/V^T feature-major tiles -- no on-chip input
transposes at all.

Per core, fully software-pipelined:
  - QKV runs per 512-token chunk (u0..u3); u0 is a standalone prologue
    (DMA-gated), later chunks are interleaved as PE "fillers" into the
    Act-engine-gated attention rounds so the PE never idles.
  - V returns to token-major layout via the DMA transpose engine (xbar),
    augmented with 64 ones-columns FIRST: the A@V matmul then yields the
    softmax denominator replicated on PSUM partitions 0:64 (base 0, where
    the custom-DVE reciprocal works) and O^T on partitions 64:128.
  - causal attention per head: scores^T = K^T-block @ Q^T (Q zero-padded
    to 128 contraction rows), exp without max-subtraction (logits are
    small and bounded).  The (head, key-block) stream runs scores one
    unit ahead of the A@V consumers, ACROSS head boundaries, so neither
    the PE nor the Activation engine ever drains.
  - row-parallel output projection with its 256 rows of c_proj_w,
    interleaved one query-block per head into the NEXT attention round,
    written as a full-shape bf16 partial [2048, 1024].
Host sums the 4 partials per batch (row-parallel all-reduce) + bias.

Precision: Q/K projections run in fp8e4m3 with the DoubleRow perf mode
(two 128-row contraction tiles per pass, 2x PE rate); V projection and
everything downstream run in bf16 (an fp8 V feeds the output linearly and
alone pushes the error to ~4e-2).  Measured end-to-end error 1.45e-2
relative to the fp32 reference's absmax, vs the 2e-2 gate.
"""

import sys

for _p in ("/opt/trn_rl_repo",):
    if _p not in sys.path:
        sys.path.insert(0, _p)

import numpy as np
import ml_dtypes

import concourse.bass as bass
import concourse.mybir as mybir
import concourse.tile as tile
from concourse import bacc
from concourse.bass_utils import run_bass_kernel_spmd

P = 128
B, S, D, H, HD = 2, 2048, 1024, 16, 64
KO = D // P            # 8 contraction blocks
NH = 4                 # heads per core
NF = 6                 # feature blocks: q01 q23 k01 k23 v01 v23
QC = 512               # query chunk
NQC = S // QC          # 4
NKB = S // P           # 16 key blocks
SCALE = 1.0 / float(HD) ** 0.5
N_CORES = 8
N_WARM = 36
# Q/K run in fp8e4m3 with DoubleRow (2 contraction-tiles per pass).
# c_attn_w's Q/K columns are pre-scaled by 64 on the host so the
# ~N(0, 0.02) entries sit in fp8's normal range; Q/K come out x64 and the
# x4096 on the scores is folded into the exp scale.  V stays bf16 at true
# scale, so w_proj needs no compensation.
WSCALE = 64.0
EXP_SCALE = SCALE / (WSCALE * WSCALE)

f32 = mybir.dt.float32
bf16 = mybir.dt.bfloat16
fp8 = mybir.dt.float8e4
EXP = mybir.ActivationFunctionType.Exp
ADD = mybir.AluOpType.add
MULT = mybir.AluOpType.mult


def _build_program(debug=False):
    nc = bacc.Bacc(None, target_bir_lowering=False)

    x_d = nc.dram_tensor("x_t", [D, S], fp8, kind="ExternalInput")
    xb_d = nc.dram_tensor("x_tb", [D, S], bf16, kind="ExternalInput")
    wq_d = nc.dram_tensor("w_qkv", [D, 4 * P], fp8, kind="ExternalInput")
    wv_d = nc.dram_tensor("w_v", [D, 2 * P], bf16, kind="ExternalInput")
    bq_d = nc.dram_tensor("b_qkv", [NF * P], f32, kind="ExternalInput")
    wp_d = nc.dram_tensor("w_proj", [2 * P, D], bf16, kind="ExternalInput")
    out_d = nc.dram_tensor("out", [S, D], bf16, kind="ExternalOutput")
    if debug:
        dbg_d = {
            nm: nc.dram_tensor(nm, [P, S], bf16, kind="ExternalOutput")
            for nm in ("d_qpad0", "d_qpad1", "d_kt0", "d_vt0", "d_vaug0", "d_xt0")
        }
        dbg_d["d_po00"] = nc.dram_tensor("d_po00", [P, QC], f32, kind="ExternalOutput")
        dbg_d["d_rbs00"] = nc.dram_tensor("d_rbs00", [HD, QC], f32, kind="ExternalOutput")
        dbg_d["d_atn0"] = nc.dram_tensor("d_atn0", [P, QC], bf16, kind="ExternalOutput")

    with tile.TileContext(nc) as tc:
        with (
            tc.tile_pool(name="const", bufs=1) as const,
            tc.tile_pool(name="pt", bufs=6) as pt_pool,
            tc.tile_pool(name="atn", bufs=2) as atn_pool,
            tc.tile_pool(name="outp", bufs=4) as out_pool,
            tc.tile_pool(name="small", bufs=3) as small_pool,
            tc.tile_pool(name="ps_mm", bufs=2, space="PSUM") as ps_mm,
            tc.tile_pool(name="ps_s", bufs=4, space="PSUM") as ps_s,
            tc.tile_pool(name="ps_o", bufs=2, space="PSUM") as ps_o,
        ):
            # ---- input DMAs, priority order: what the first QKV chunk
            # needs arrives first; later x^T chunks and proj weights are
            # issued after the u0 compute is emitted so they don't steal
            # queue bandwidth from the critical u0 transfer.
            wq_sb = const.tile([P, KO, 4 * P], fp8)
            wqr = wq_d.rearrange("(ko p) f -> p ko f", p=P)
            wv_sb = const.tile([P, KO, 2 * P], bf16)
            wvr = wv_d.rearrange("(ko p) f -> p ko f", p=P)
            xt = const.tile([P, KO, S], fp8)
            xr = x_d.rearrange("(ko p) t -> p ko t", p=P)
            xtb = const.tile([P, KO, S], bf16)
            xbr = xb_d.rearrange("(ko p) t -> p ko t", p=P)
            # V path (bf16, 2x the bytes) first: it gates V_aug and attn0
            nc.sync.dma_start(wv_sb[:], wvr[:])
            nc.sync.dma_start(xtb[:, :, :QC], xbr[:, :, :QC])
            nc.sync.dma_start(wq_sb[:], wqr[:])
            nc.sync.dma_start(xt[:, :, :QC], xr[:, :, :QC])
            bq_sb = const.tile([P, NF], f32)
            nc.sync.dma_start(bq_sb[:], bq_d.rearrange("(c p) -> p c", p=P))
            nc.sync.dma_start(xtb[:, :, QC : 2 * QC], xbr[:, :, QC : 2 * QC])
            nc.sync.dma_start(xt[:, :, QC : 2 * QC], xr[:, :, QC : 2 * QC])
            wp_sb = const.tile([P, 2, D], bf16)

            # ---- constants ----
            # mask[k, q] = 1.0 if k <= q else 0.0 (diagonal 128-blocks)
            mask_f = const.tile([P, P], f32)
            nc.gpsimd.memset(mask_f[:], 1.0)
            nc.gpsimd.affine_select(
                out=mask_f[:], in_=mask_f[:],
                compare_op=mybir.AluOpType.is_ge, fill=0.0,
                base=0, pattern=[[1, P]], channel_multiplier=-1,
            )
            mask = const.tile([P, P], bf16)
            nc.vector.tensor_copy(mask[:], mask_f[:])

            # PE p-state warmup during the input DMAs
            warm = const.tile([P, P], bf16)
            nc.gpsimd.memset(warm[:], 0.25)
            for _ in range(N_WARM):
                psw = ps_s.tile([P, QC], f32, tag="s", name="psw")
                nc.tensor.matmul(psw[:, :P], warm[:], warm[:], start=True, stop=True)

            # ---- persistent QKV^T tiles ----
            # Q^T per head, zero-padded to 128 contraction rows (even heads
            # live in rows 0:64, odd in 64:128 -- matching the stacked K^T
            # pair tiles, so a full-128-partition matmul contracts exactly).
            qpad = [const.tile([P, S], bf16, tag=f"qp{h}", name=f"qp{h}")
                    for h in range(NH)]
            kt = [const.tile([P, S], bf16, tag=f"kt{r}", name=f"kt{r}")
                  for r in range(2)]
            vt = [const.tile([P, S], bf16, tag=f"vt{r}", name=f"vt{r}")
                  for r in range(2)]
            # V_aug[token, 0:64] = 1.0 (denominator columns, base 0 for the
            # custom-DVE reciprocal); [token, 64:128] = V features.
            vaug = [const.tile([P, NKB, P], bf16, tag=f"va{h}", name=f"va{h}")
                    for h in range(NH)]
            for h in range(NH):
                pad = qpad[h][HD:, :] if h % 2 == 0 else qpad[h][:HD, :]
                nc.gpsimd.memset(pad, 0.0)
                nc.gpsimd.memset(vaug[h][:, :, :HD], 1.0)

            def qkv_fcs(u, fcs):
                ucs = slice(u * QC, (u + 1) * QC)
                for fc in fcs:
                    ps = ps_mm.tile([P, QC], f32, tag="mm", name="psq")
                    if fc < 4:   # Q/K: fp8 DoubleRow, 2 K-tiles per pass
                        for kp in range(KO // 2):
                            nc.tensor.matmul(
                                ps[:],
                                wq_sb[:, 2 * kp : 2 * kp + 2, fc * P : (fc + 1) * P],
                                xt[:, 2 * kp : 2 * kp + 2, ucs],
                                start=(kp == 0),
                                stop=(kp == KO // 2 - 1),
                                perf_mode=mybir.MatmulPerfMode.DoubleRow,
                            )
                    else:        # V: bf16 (fp8 V dominates output error)
                        for ko in range(KO):
                            nc.tensor.matmul(
                                ps[:],
                                wv_sb[:, ko, (fc - 4) * P : (fc - 3) * P],
                                xtb[:, ko, ucs],
                                start=(ko == 0),
                                stop=(ko == KO - 1),
                            )
                    if fc < 2:  # q01 / q23 -> per-head padded Q^T
                        h0, h1 = 2 * fc, 2 * fc + 1
                        nc.vector.tensor_scalar(
                            qpad[h0][:HD, ucs], ps[:HD],
                            bq_sb[:HD, fc : fc + 1], None, ADD)
                        nc.vector.tensor_scalar(
                            qpad[h1][HD:, ucs], ps[HD:],
                            bq_sb[HD:, fc : fc + 1], None, ADD)
                    else:
                        dst = kt[fc - 2] if fc < 4 else vt[fc - 4]
                        nc.vector.tensor_scalar(
                            dst[:, ucs], ps[:],
                            bq_sb[:, fc : fc + 1], None, ADD)

            def vaug_u(u):
                # V^T [64 feats, 512 tokens] -> token-major via xbar DMA
                for h in range(NH):
                    pr, hl = divmod(h, 2)
                    nc.sync.dma_start_transpose(
                        vaug[h][:, u * 4 : (u + 1) * 4, HD:],
                        vt[pr][hl * HD : (hl + 1) * HD, u * QC : (u + 1) * QC],
                    )

            def proj_qb_group(qc, atns, qb, pool=None, tag="mm"):
                pool = pool if pool is not None else ps_mm
                pps = [pool.tile([P, QC], f32, tag=tag, name="pp")
                       for _ in range(2)]
                for pr in range(2):
                    for nck in range(2):
                        nc.tensor.matmul(
                            pps[nck][:],
                            atns[pr][:, qb * P : (qb + 1) * P],
                            wp_sb[:, pr, nck * QC : (nck + 1) * QC],
                            start=(pr == 0), stop=(pr == 1),
                        )
                for nck in range(2):
                    ot = out_pool.tile([P, QC], bf16, tag="ot", name="ot")
                    nc.vector.tensor_copy(ot[:], pps[nck][:])
                    row = qc * QC + qb * P
                    nc.sync.dma_start(
                        out_d[row : row + P, nck * QC : (nck + 1) * QC],
                        ot[:],
                    )

            def attn_qc(qc, atns, proj_prev=None, fillers=None, dbg=None):
                """One attention round, (head, key-block) software-pipelined:
                scores run one unit ahead of A@V across head boundaries.
                After each head's normalize: the previous round's projection
                query-block, then that head's PE filler (QKV chunk work)."""
                qcs = slice(qc * QC, (qc + 1) * QC)
                pos = [None] * NH

                def emit_scores(h, kind, kb):
                    pr = h // 2
                    ps = ps_s.tile([P, QC], f32, tag="s", name="pss")
                    pt = pt_pool.tile([P, QC], bf16, tag="pt", name="pt")
                    if kind == "o":
                        nc.tensor.matmul(
                            ps[:], kt[pr][:, kb * P : (kb + 1) * P],
                            qpad[h][:, qcs], start=True, stop=True)
                        nc.scalar.activation(pt[:], ps[:], EXP, scale=EXP_SCALE)
                    else:
                        lo = (kb - qc * 4) * P
                        nc.tensor.matmul(
                            ps[:, lo:], kt[pr][:, kb * P : (kb + 1) * P],
                            qpad[h][:, qc * QC + lo : (qc + 1) * QC],
                            start=True, stop=True)
                        nc.scalar.activation(pt[:, lo:], ps[:, lo:], EXP,
                                             scale=EXP_SCALE)
                        nc.vector.tensor_tensor(
                            pt[:, lo : lo + P], pt[:, lo : lo + P],
                            mask[:], MULT)
                    return pt

                def emit_av(h, kind, kb, pt, first, last):
                    if pos[h] is None:
                        pos[h] = ps_o.tile([P, QC], f32, tag="po", name="po")
                    po = pos[h]
                    if kind == "o":
                        nc.tensor.matmul(po[:], vaug[h][:, kb, :], pt[:],
                                         start=first, stop=last)
                    else:
                        lo = (kb - qc * 4) * P
                        nc.tensor.matmul(po[:, lo:], vaug[h][:, kb, :],
                                         pt[:, lo:], start=first, stop=last)

                def finish_head(h):
                    pr, hl = divmod(h, 2)
                    po = pos[h]
                    # denominator arrives replicated on po[0:64] (base 0)
                    rbs = small_pool.tile([HD, QC], f32, tag="rbs", name="rbs")
                    nc.vector.reciprocal_approx_fast(out=rbs[:], in_=po[:HD, :])
                    if dbg is not None and h == 0:
                        pocp = out_pool.tile([P, QC], f32, tag="dbg", name="dbg")
                        nc.vector.tensor_copy(pocp[:], po[:])
                        nc.sync.dma_start(dbg["d_po00"][:], pocp[:])
                        nc.sync.dma_start(dbg["d_rbs00"][:], rbs[:])
                    nc.vector.tensor_tensor(
                        atns[pr][hl * HD : (hl + 1) * HD, :],
                        po[HD:, :], rbs[:], MULT,
                    )
                    if proj_prev is not None:
                        proj_qb_group(proj_prev[0], proj_prev[1], qb=h)
                    if fillers is not None and fillers[h] is not None:
                        fillers[h]()

                stream = []
                for h in range(NH):
                    units = [("o", kb) for kb in range(qc * 4)] + \
                            [("d", qc * 4 + j) for j in range(4)]
                    for i, (kind, kb) in enumerate(units):
                        stream.append((h, kind, kb, i == 0,
                                       i == len(units) - 1))

                pending = None
                for h, kind, kb, first, last in stream:
                    pt = emit_scores(h, kind, kb)
                    if pending is not None:
                        ph, pk, pkb, ppt, pf, pl = pending
                        emit_av(ph, pk, pkb, ppt, pf, pl)
                        if pl:
                            finish_head(ph)
                    pending = (h, kind, kb, pt, first, last)
                ph, pk, pkb, ppt, pf, pl = pending
                emit_av(ph, pk, pkb, ppt, pf, pl)
                finish_head(ph)

            def mk_atns():
                return [atn_pool.tile([P, QC], bf16, tag=f"atn{r}",
                                      name=f"atn{r}") for r in range(2)]

            # ---- schedule ----
            # V projections first (their bf16 inputs arrive first and they
            # gate V_aug + attn0); Q/K fp8 matmuls follow.
            qkv_fcs(0, (4, 5, 0, 1, 2, 3))
            # u2 issues before the sequencer-blocking vaug transposes (its
            # consumers in the attn1 fillers come up at ~40us); u3 + wp can
            # wait behind them.
            nc.sync.dma_start(xtb[:, :, 2 * QC : 3 * QC],
                              xbr[:, :, 2 * QC : 3 * QC])
            nc.sync.dma_start(xt[:, :, 2 * QC : 3 * QC],
                              xr[:, :, 2 * QC : 3 * QC])
            vaug_u(0)
            nc.sync.dma_start(xtb[:, :, 3 * QC :], xbr[:, :, 3 * QC :])
            nc.sync.dma_start(xt[:, :, 3 * QC :], xr[:, :, 3 * QC :])
            nc.sync.dma_start(wp_sb[:], wp_d.rearrange("(c p) d -> p c d", p=P))
            a0 = mk_atns()
            attn_qc(0, a0, fillers=[
                lambda: qkv_fcs(1, (0, 1)),
                lambda: qkv_fcs(1, (2, 3)),
                lambda: qkv_fcs(1, (4, 5)),
                lambda: vaug_u(1),
            ], dbg=(dbg_d if debug else None))
            if debug:
                nc.sync.dma_start(dbg_d["d_atn0"][:], a0[0][:])
            a1 = mk_atns()
            attn_qc(1, a1, proj_prev=(0, a0), fillers=[
                None,
                lambda: qkv_fcs(2, (4, 5)),
                lambda: qkv_fcs(2, (0, 1, 2, 3)),
                lambda: vaug_u(2),
            ])
            a2 = mk_atns()
            attn_qc(2, a2, proj_prev=(1, a1), fillers=[
                lambda: qkv_fcs(3, (4, 5)),
                lambda: qkv_fcs(3, (0, 1)),
                lambda: qkv_fcs(3, (2, 3)),
                lambda: vaug_u(3),
            ])
            a3 = mk_atns()
            attn_qc(3, a3, proj_prev=(2, a2))
            # final projection runs in the (now idle) score pool: 4-deep
            # PSUM rotation so the evict casts pipeline behind the matmuls
            for qb in range(4):
                proj_qb_group(3, a3, qb, pool=ps_s, tag="s")

            if debug:
                nc.sync.dma_start(dbg_d["d_qpad0"][:], qpad[0][:])
                nc.sync.dma_start(dbg_d["d_qpad1"][:], qpad[1][:])
                nc.sync.dma_start(dbg_d["d_kt0"][:], kt[0][:])
                nc.sync.dma_start(dbg_d["d_vt0"][:], vt[0][:])
                nc.sync.dma_start(
                    dbg_d["d_vaug0"][:],
                    vaug[0].rearrange("p a b -> p (a b)"),
                )
                nc.sync.dma_start(dbg_d["d_xt0"][:], xt[:, 0, :])

    nc.compile()
    return nc


_CACHE = {}


def get_program():
    if "p" not in _CACHE:
        _CACHE["p"] = _build_program()
    return _CACHE["p"]


def make_in_maps(hidden_states, c_attn_w, c_attn_b, c_proj_w):
    x = np.asarray(hidden_states, dtype=np.float32).reshape(B, S, D)
    wa = np.asarray(c_attn_w, dtype=np.float32)
    ba = np.asarray(c_attn_b, dtype=np.float32)
    wp = np.asarray(c_proj_w, dtype=np.float32)
    bf = ml_dtypes.bfloat16

    f8 = ml_dtypes.float8_e4m3
    xts = [np.ascontiguousarray(x[b].T).astype(f8) for b in range(B)]
    xtbs = [np.ascontiguousarray(x[b].T).astype(bf) for b in range(B)]
    in_maps = []
    for c in range(N_CORES):
        b, g = divmod(c, 4)
        w_blocks, b_blocks = [], []
        for m in range(3):          # q, k, v
            base = m * D + g * 256
            for half in range(2):   # heads (0,1) then (2,3) of the group
                w_blocks.append(wa[:, base + half * P : base + (half + 1) * P])
                b_blocks.append(ba[base + half * P : base + (half + 1) * P])
        # block order q01 q23 k01 k23 | v01 v23; Q/K x64 pre-scale for fp8
        w_qkv = np.ascontiguousarray(
            np.concatenate(w_blocks[:4], axis=1) * WSCALE).astype(f8)
        w_v = np.ascontiguousarray(
            np.concatenate(w_blocks[4:], axis=1)).astype(bf)
        b_qkv = np.ascontiguousarray(np.concatenate(
            [bb * WSCALE for bb in b_blocks[:4]] + b_blocks[4:]))
        w_proj = np.ascontiguousarray(wp[g * 256 : (g + 1) * 256, :]).astype(bf)
        in_maps.append({
            "x_t": xts[b],
            "x_tb": xtbs[b],
            "w_qkv": w_qkv,
            "w_v": w_v,
            "b_qkv": b_qkv,
            "w_proj": w_proj,
        })
    return in_maps


def kernel(hidden_states, c_attn_w, c_attn_b, c_proj_w, c_proj_b):
    nc = get_program()
    in_maps = make_in_maps(hidden_states, c_attn_w, c_attn_b, c_proj_w)
    res = run_bass_kernel_spmd(nc, in_maps, list(range(N_CORES)))
    bias = np.asarray(c_proj_b, dtype=np.float32)[None, :]
    outs = []
    for b in range(B):
        acc = res.results[b * 4]["out"].astype(np.float32)
        for g in range(1, 4):
            acc = acc + res.results[b * 4 + g]["out"].astype(np.float32)
        outs.append(acc + bias)
    return np.stack(outs).reshape(B, S, D).astype(np.float32)


if __name__ == "__main__":
    rng = np.random.default_rng(0)
    hs = rng.standard_normal((B, S, D), dtype=np.float32)
    wa = rng.standard_normal((D, 3 * D), dtype=np.float32) * 0.02
    ba = rng.standard_normal((3 * D,), dtype=np.float32) * 0.02
    wp = rng.standard_normal((D, D), dtype=np.float32) * 0.02
    bp = rng.standard_normal((D,), dtype=np.float32) * 0.02
    out = kernel(hs, wa, ba, wp, bp)
    print("out", out.shape, out.dtype, float(np.abs(out).max()))


# revision 49
# speedup vs baseline: 1.1809x; 1.0181x over previous
"""GPT2-style fused attention (DecisionTransformer) on 8 Trainium2 NeuronCores.

Sharding: batch x head-group.  Core c = b*4 + g handles batch b and heads
4g..4g+3.  The host pre-transposes each batch's activations to x^T [D, S]
and casts to bf16 (layout prep during sharding), so the QKV projection
directly produces Q^T/K^T/V^T feature-major tiles -- no on-chip input
transposes at all.

Per core, fully software-pipelined:
  - QKV runs per 512-token chunk (u0..u3); u0 is a standalone prologue
    (DMA-gated), later chunks are interleaved as PE "fillers" into the
    Act-engine-gated attention rounds so the PE never idles.
  - V returns to token-major layout via the DMA transpose engine (xbar),
    augmented with 64 ones-columns FIRST: the A@V matmul then yields the
    softmax denominator replicated on PSUM partitions 0:64 (base 0, where
    the custom-DVE reciprocal works) and O^T on partitions 64:128.
  - causal attention per head: scores^T = K^T-block @ Q^T (Q zero-padded
    to 128 contraction rows), exp without max-subtraction (logits are
    small and bounded).  The (head, key-block) stream runs scores one
    unit ahead of the A@V consumers, ACROSS head boundaries, so neither
    the PE nor the Activation engine ever drains.
  - row-parallel output projection with its 256 rows of c_proj_w,
    interleaved one query-block per head into the NEXT attention round,
    written as a full-shape bf16 partial [2048, 1024].
Host sums the 4 partials per batch (row-parallel all-reduce) + bias.

Precision: Q/K projections run in fp8e4m3 with the DoubleRow perf mode
(two 128-row contraction tiles per pass, 2x PE rate); V projection and
everything downstream run in bf16 (an fp8 V feeds the output linearly and
alone pushes the error to ~4e-2).  Measured end-to-end error 1.45e-2
relative to the fp32 reference's absmax, vs the 2e-2 gate.
"""

import sys

for _p in ("/opt/trn_rl_repo",):
    if _p not in sys.path:
        sys.path.insert(0, _p)

import numpy as np
import ml_dtypes

import concourse.bass as bass
import concourse.mybir as mybir
import concourse.tile as tile
from concourse import bacc
from concourse.bass_utils import run_bass_kernel_spmd

P = 128
B, S, D, H, HD = 2, 2048, 1024, 16, 64
KO = D // P            # 8 contraction blocks
NH = 4                 # heads per core
NF = 6                 # feature blocks: q01 q23 k01 k23 v01 v23
QC = 512               # query chunk
NQC = S // QC          # 4
NKB = S // P           # 16 key blocks
SCALE = 1.0 / float(HD) ** 0.5
N_CORES = 8
N_WARM = 66
# Q/K run in fp8e4m3 with DoubleRow (2 contraction-tiles per pass).
# c_attn_w's Q/K columns are pre-scaled by 64 on the host so the
# ~N(0, 0.02) entries sit in fp8's normal range; Q/K come out x64 and the
# x4096 on the scores is folded into the exp scale.  V stays bf16 at true
# scale, so w_proj needs no compensation.
WSCALE = 64.0
EXP_SCALE = SCALE / (WSCALE * WSCALE)

f32 = mybir.dt.float32
bf16 = mybir.dt.bfloat16
fp8 = mybir.dt.float8e4
EXP = mybir.ActivationFunctionType.Exp
ADD = mybir.AluOpType.add
MULT = mybir.AluOpType.mult


def _build_program(debug=False):
    nc = bacc.Bacc(None, target_bir_lowering=False)

    x_d = nc.dram_tensor("x_t", [D, S], fp8, kind="ExternalInput")
    xb_d = nc.dram_tensor("x_tb", [D, S], bf16, kind="ExternalInput")
    wq_d = nc.dram_tensor("w_qkv", [D, 4 * P], fp8, kind="ExternalInput")
    wv_d = nc.dram_tensor("w_v", [D, 2 * P], bf16, kind="ExternalInput")
    bq_d = nc.dram_tensor("b_qkv", [NF * P], f32, kind="ExternalInput")
    wp_d = nc.dram_tensor("w_proj", [2 * P, D], bf16, kind="ExternalInput")
    out_d = nc.dram_tensor("out", [S, D], bf16, kind="ExternalOutput")
    if debug:
        dbg_d = {
            nm: nc.dram_tensor(nm, [P, S], bf16, kind="ExternalOutput")
            for nm in ("d_qpad0", "d_qpad1", "d_kt0", "d_vt0", "d_vaug0", "d_xt0")
        }
        dbg_d["d_po00"] = nc.dram_tensor("d_po00", [P, QC], f32, kind="ExternalOutput")
        dbg_d["d_rbs00"] = nc.dram_tensor("d_rbs00", [HD, QC], f32, kind="ExternalOutput")
        dbg_d["d_atn0"] = nc.dram_tensor("d_atn0", [P, QC], bf16, kind="ExternalOutput")

    with tile.TileContext(nc) as tc:
        with (
            tc.tile_pool(name="const", bufs=1) as const,
            tc.tile_pool(name="pt", bufs=6) as pt_pool,
            tc.tile_pool(name="atn", bufs=2) as atn_pool,
            tc.tile_pool(name="outp", bufs=4) as out_pool,
            tc.tile_pool(name="small", bufs=3) as small_pool,
            tc.tile_pool(name="ps_mm", bufs=2, space="PSUM") as ps_mm,
            tc.tile_pool(name="ps_s", bufs=4, space="PSUM") as ps_s,
            tc.tile_pool(name="ps_o", bufs=2, space="PSUM") as ps_o,
        ):
            # ---- input DMAs, priority order: what the first QKV chunk
            # needs arrives first; later x^T chunks and proj weights are
            # issued after the u0 compute is emitted so they don't steal
            # queue bandwidth from the critical u0 transfer.
            wq_sb = const.tile([P, KO, 4 * P], fp8)
            wqr = wq_d.rearrange("(ko p) f -> p ko f", p=P)
            wv_sb = const.tile([P, KO, 2 * P], bf16)
            wvr = wv_d.rearrange("(ko p) f -> p ko f", p=P)
            xt = const.tile([P, KO, S], fp8)
            xr = x_d.rearrange("(ko p) t -> p ko t", p=P)
            xtb = const.tile([P, KO, S], bf16)
            xbr = xb_d.rearrange("(ko p) t -> p ko t", p=P)
            # V path (bf16, 2x the bytes) first: it gates V_aug and attn0
            nc.sync.dma_start(wv_sb[:], wvr[:])
            nc.sync.dma_start(xtb[:, :, :QC], xbr[:, :, :QC])
            nc.sync.dma_start(wq_sb[:], wqr[:])
            nc.sync.dma_start(xt[:, :, :QC], xr[:, :, :QC])
            bq_sb = const.tile([P, NF], f32)
            nc.sync.dma_start(bq_sb[:], bq_d.rearrange("(c p) -> p c", p=P))
            nc.sync.dma_start(xtb[:, :, QC : 2 * QC], xbr[:, :, QC : 2 * QC])
            nc.sync.dma_start(xt[:, :, QC : 2 * QC], xr[:, :, QC : 2 * QC])
            wp_sb = const.tile([P, 2, D], bf16)

            # ---- constants ----
            # mask[k, q] = 1.0 if k <= q else 0.0 (diagonal 128-blocks)
            mask_f = const.tile([P, P], f32)
            nc.gpsimd.memset(mask_f[:], 1.0)
            nc.gpsimd.affine_select(
                out=mask_f[:], in_=mask_f[:],
                compare_op=mybir.AluOpType.is_ge, fill=0.0,
                base=0, pattern=[[1, P]], channel_multiplier=-1,
            )
            mask = const.tile([P, P], bf16)
            nc.vector.tensor_copy(mask[:], mask_f[:])

            # PE p-state warmup during the input DMAs
            warm = const.tile([P, P], bf16)
            nc.gpsimd.memset(warm[:], 0.25)
            for _ in range(N_WARM):
                psw = ps_s.tile([P, QC], f32, tag="s", name="psw")
                nc.tensor.matmul(psw[:, :P], warm[:], warm[:], start=True, stop=True)

            # ---- persistent QKV^T tiles ----
            # Q^T per head, zero-padded to 128 contraction rows (even heads
            # live in rows 0:64, odd in 64:128 -- matching the stacked K^T
            # pair tiles, so a full-128-partition matmul contracts exactly).
            qpad = [const.tile([P, S], bf16, tag=f"qp{h}", name=f"qp{h}")
                    for h in range(NH)]
            kt = [const.tile([P, S], bf16, tag=f"kt{r}", name=f"kt{r}")
                  for r in range(2)]
            vt = [const.tile([P, S], bf16, tag=f"vt{r}", name=f"vt{r}")
                  for r in range(2)]
            # V_aug[token, 0:64] = 1.0 (denominator columns, base 0 for the
            # custom-DVE reciprocal); [token, 64:128] = V features.
            vaug = [const.tile([P, NKB, P], bf16, tag=f"va{h}", name=f"va{h}")
                    for h in range(NH)]
            for h in range(NH):
                pad = qpad[h][HD:, :] if h % 2 == 0 else qpad[h][:HD, :]
                nc.gpsimd.memset(pad, 0.0)
                nc.gpsimd.memset(vaug[h][:, :, :HD], 1.0)

            def qkv_fcs(u, fcs):
                ucs = slice(u * QC, (u + 1) * QC)
                for fc in fcs:
                    ps = ps_mm.tile([P, QC], f32, tag="mm", name="psq")
                    if fc < 4:   # Q/K: fp8 DoubleRow, 2 K-tiles per pass
                        for kp in range(KO // 2):
                            nc.tensor.matmul(
                                ps[:],
                                wq_sb[:, 2 * kp : 2 * kp + 2, fc * P : (fc + 1) * P],
                                xt[:, 2 * kp : 2 * kp + 2, ucs],
                                start=(kp == 0),
                                stop=(kp == KO // 2 - 1),
                                perf_mode=mybir.MatmulPerfMode.DoubleRow,
                            )
                    else:        # V: bf16 (fp8 V dominates output error)
                        for ko in range(KO):
                            nc.tensor.matmul(
                                ps[:],
                                wv_sb[:, ko, (fc - 4) * P : (fc - 3) * P],
                                xtb[:, ko, ucs],
                                start=(ko == 0),
                                stop=(ko == KO - 1),
                            )
                    if fc < 2:  # q01 / q23 -> per-head padded Q^T
                        h0, h1 = 2 * fc, 2 * fc + 1
                        nc.vector.tensor_scalar(
                            qpad[h0][:HD, ucs], ps[:HD],
                            bq_sb[:HD, fc : fc + 1], None, ADD)
                        nc.vector.tensor_scalar(
                            qpad[h1][HD:, ucs], ps[HD:],
                            bq_sb[HD:, fc : fc + 1], None, ADD)
                    else:
                        dst = kt[fc - 2] if fc < 4 else vt[fc - 4]
                        nc.vector.tensor_scalar(
                            dst[:, ucs], ps[:],
                            bq_sb[:, fc : fc + 1], None, ADD)

            def vaug_u(u):
                # V^T [64 feats, 512 tokens] -> token-major via xbar DMA
                for h in range(NH):
                    pr, hl = divmod(h, 2)
                    nc.sync.dma_start_transpose(
                        vaug[h][:, u * 4 : (u + 1) * 4, HD:],
                        vt[pr][hl * HD : (hl + 1) * HD, u * QC : (u + 1) * QC],
                    )

            def proj_qb_group(qc, atns, qb, pool=None, tag="mm"):
                pool = pool if pool is not None else ps_mm
                pps = [pool.tile([P, QC], f32, tag=tag, name="pp")
                       for _ in range(2)]
                for pr in range(2):
                    for nck in range(2):
                        nc.tensor.matmul(
                            pps[nck][:],
                            atns[pr][:, qb * P : (qb + 1) * P],
                            wp_sb[:, pr, nck * QC : (nck + 1) * QC],
                            start=(pr == 0), stop=(pr == 1),
                        )
                for nck in range(2):
                    ot = out_pool.tile([P, QC], bf16, tag="ot", name="ot")
                    nc.vector.tensor_copy(ot[:], pps[nck][:])
                    row = qc * QC + qb * P
                    nc.sync.dma_start(
                        out_d[row : row + P, nck * QC : (nck + 1) * QC],
                        ot[:],
                    )

            def attn_qc(qc, atns, proj_prev=None, fillers=None, dbg=None):
                """One attention round, (head, key-block) software-pipelined:
                scores run one unit ahead of A@V across head boundaries.
                After each head's normalize: the previous round's projection
                query-block, then that head's PE filler (QKV chunk work)."""
                qcs = slice(qc * QC, (qc + 1) * QC)
                pos = [None] * NH

                def emit_scores(h, kind, kb):
                    pr = h // 2
                    ps = ps_s.tile([P, QC], f32, tag="s", name="pss")
                    pt = pt_pool.tile([P, QC], bf16, tag="pt", name="pt")
                    if kind == "o":
                        nc.tensor.matmul(
                            ps[:], kt[pr][:, kb * P : (kb + 1) * P],
                            qpad[h][:, qcs], start=True, stop=True)
                        nc.scalar.activation(pt[:], ps[:], EXP, scale=EXP_SCALE)
                    else:
                        lo = (kb - qc * 4) * P
                        nc.tensor.matmul(
                            ps[:, lo:], kt[pr][:, kb * P : (kb + 1) * P],
                            qpad[h][:, qc * QC + lo : (qc + 1) * QC],
                            start=True, stop=True)
                        nc.scalar.activation(pt[:, lo:], ps[:, lo:], EXP,
                                             scale=EXP_SCALE)
                        nc.vector.tensor_tensor(
                            pt[:, lo : lo + P], pt[:, lo : lo + P],
                            mask[:], MULT)
                    return pt

                def emit_av(h, kind, kb, pt, first, last):
                    if pos[h] is None:
                        pos[h] = ps_o.tile([P, QC], f32, tag="po", name="po")
                    po = pos[h]
                    if kind == "o":
                        nc.tensor.matmul(po[:], vaug[h][:, kb, :], pt[:],
                                         start=first, stop=last)
                    else:
                        lo = (kb - qc * 4) * P
                        nc.tensor.matmul(po[:, lo:], vaug[h][:, kb, :],
                                         pt[:, lo:], start=first, stop=last)

                def finish_head(h):
                    pr, hl = divmod(h, 2)
                    po = pos[h]
                    # denominator arrives replicated on po[0:64] (base 0)
                    rbs = small_pool.tile([HD, QC], f32, tag="rbs", name="rbs")
                    nc.vector.reciprocal_approx_fast(out=rbs[:], in_=po[:HD, :])
                    if dbg is not None and h == 0:
                        pocp = out_pool.tile([P, QC], f32, tag="dbg", name="dbg")
                        nc.vector.tensor_copy(pocp[:], po[:])
                        nc.sync.dma_start(dbg["d_po00"][:], pocp[:])
                        nc.sync.dma_start(dbg["d_rbs00"][:], rbs[:])
                    nc.vector.tensor_tensor(
                        atns[pr][hl * HD : (hl + 1) * HD, :],
                        po[HD:, :], rbs[:], MULT,
                    )
                    if proj_prev is not None:
                        proj_qb_group(proj_prev[0], proj_prev[1], qb=h)
                    if fillers is not None and fillers[h] is not None:
                        fillers[h]()

                stream = []
                for h in range(NH):
                    units = [("o", kb) for kb in range(qc * 4)] + \
                            [("d", qc * 4 + j) for j in range(4)]
                    for i, (kind, kb) in enumerate(units):
                        stream.append((h, kind, kb, i == 0,
                                       i == len(units) - 1))

                pending = None
                for h, kind, kb, first, last in stream:
                    pt = emit_scores(h, kind, kb)
                    if pending is not None:
                        ph, pk, pkb, ppt, pf, pl = pending
                        emit_av(ph, pk, pkb, ppt, pf, pl)
                        if pl:
                            finish_head(ph)
                    pending = (h, kind, kb, pt, first, last)
                ph, pk, pkb, ppt, pf, pl = pending
                emit_av(ph, pk, pkb, ppt, pf, pl)
                finish_head(ph)

            def mk_atns():
                return [atn_pool.tile([P, QC], bf16, tag=f"atn{r}",
                                      name=f"atn{r}") for r in range(2)]

            # ---- schedule ----
            # V projections first (their bf16 inputs arrive first and they
            # gate V_aug + attn0); Q/K fp8 matmuls follow.
            qkv_fcs(0, (4, 5, 0, 1, 2, 3))
            vaug_u(0)
            # deferred input DMAs: issued only after the u0/u1 critical
            # transfers have drained; needed from the attn1 fillers onward
            for u in range(2, 4):
                nc.sync.dma_start(xt[:, :, u * QC : (u + 1) * QC],
                                  xr[:, :, u * QC : (u + 1) * QC])
                nc.sync.dma_start(xtb[:, :, u * QC : (u + 1) * QC],
                                  xbr[:, :, u * QC : (u + 1) * QC])
            nc.sync.dma_start(wp_sb[:], wp_d.rearrange("(c p) d -> p c d", p=P))
            a0 = mk_atns()
            attn_qc(0, a0, fillers=[
                lambda: qkv_fcs(1, (0, 1)),
                lambda: qkv_fcs(1, (2, 3)),
                lambda: qkv_fcs(1, (4, 5)),
                lambda: vaug_u(1),
            ], dbg=(dbg_d if debug else None))
            if debug:
                nc.sync.dma_start(dbg_d["d_atn0"][:], a0[0][:])
            a1 = mk_atns()
            attn_qc(1, a1, proj_prev=(0, a0), fillers=[
                lambda: qkv_fcs(2, (0, 1)),
                lambda: qkv_fcs(2, (2, 3)),
                lambda: qkv_fcs(2, (4, 5)),
                lambda: vaug_u(2),
            ])
            a2 = mk_atns()
            attn_qc(2, a2, proj_prev=(1, a1), fillers=[
                lambda: qkv_fcs(3, (0, 1)),
                lambda: qkv_fcs(3, (2, 3)),
                lambda: qkv_fcs(3, (4, 5)),
                lambda: vaug_u(3),
            ])
            a3 = mk_atns()
            attn_qc(3, a3, proj_prev=(2, a2))
            # final projection runs in the (now idle) score pool: 4-deep
            # PSUM rotation so the evict casts pipeline behind the matmuls
            for qb in range(4):
                proj_qb_group(3, a3, qb, pool=ps_s, tag="s")

            if debug:
                nc.sync.dma_start(dbg_d["d_qpad0"][:], qpad[0][:])
                nc.sync.dma_start(dbg_d["d_qpad1"][:], qpad[1][:])
                nc.sync.dma_start(dbg_d["d_kt0"][:], kt[0][:])
                nc.sync.dma_start(dbg_d["d_vt0"][:], vt[0][:])
                nc.sync.dma_start(
                    dbg_d["d_vaug0"][:],
                    vaug[0].rearrange("p a b -> p (a b)"),
                )
                nc.sync.dma_start(dbg_d["d_xt0"][:], xt[:, 0, :])

    nc.compile()
    return nc


_CACHE = {}


def get_program():
    if "p" not in _CACHE:
        _CACHE["p"] = _build_program()
    return _CACHE["p"]


def make_in_maps(hidden_states, c_attn_w, c_attn_b, c_proj_w):
    x = np.asarray(hidden_states, dtype=np.float32).reshape(B, S, D)
    wa = np.asarray(c_attn_w, dtype=np.float32)
    ba = np.asarray(c_attn_b, dtype=np.float32)
    wp = np.asarray(c_proj_w, dtype=np.float32)
    bf = ml_dtypes.bfloat16

    f8 = ml_dtypes.float8_e4m3
    xts = [np.ascontiguousarray(x[b].T).astype(f8) for b in range(B)]
    xtbs = [np.ascontiguousarray(x[b].T).astype(bf) for b in range(B)]
    in_maps = []
    for c in range(N_CORES):
        b, g = divmod(c, 4)
        w_blocks, b_blocks = [], []
        for m in range(3):          # q, k, v
            base = m * D + g * 256
            for half in range(2):   # heads (0,1) then (2,3) of the group
                w_blocks.append(wa[:, base + half * P : base + (half + 1) * P])
                b_blocks.append(ba[base + half * P : base + (half + 1) * P])
        # block order q01 q23 k01 k23 | v01 v23; Q/K x64 pre-scale for fp8
        w_qkv = np.ascontiguousarray(
            np.concatenate(w_blocks[:4], axis=1) * WSCALE).astype(f8)
        w_v = np.ascontiguousarray(
            np.concatenate(w_blocks[4:], axis=1)).astype(bf)
        b_qkv = np.ascontiguousarray(np.concatenate(
            [bb * WSCALE for bb in b_blocks[:4]] + b_blocks[4:]))
        w_proj = np.ascontiguousarray(wp[g * 256 : (g + 1) * 256, :]).astype(bf)
        in_maps.append({
            "x_t": xts[b],
            "x_tb": xtbs[b],
            "w_qkv": w_qkv,
            "w_v": w_v,
            "b_qkv": b_qkv,
            "w_proj": w_proj,
        })
    return in_maps


def kernel(hidden_states, c_attn_w, c_attn_b, c_proj_w, c_proj_b):
    nc = get_program()
    in_maps = make_in_maps(hidden_states, c_attn_w, c_attn_b, c_proj_w)
    res = run_bass_kernel_spmd(nc, in_maps, list(range(N_CORES)))
    bias = np.asarray(c_proj_b, dtype=np.float32)[None, :]
    outs = []
    for b in range(B):
        acc = res.results[b * 4]["out"].astype(np.float32)
        for g in range(1, 4):
            acc = acc + res.results[b * 4 + g]["out"].astype(np.float32)
        outs.append(acc + bias)
    return np.stack(outs).reshape(B, S, D).astype(np.float32)


if __name__ == "__main__":
    rng = np.random.default_rng(0)
    hs = rng.standard_normal((B, S, D), dtype=np.float32)
    wa = rng.standard_normal((D, 3 * D), dtype=np.float32) * 0.02
    ba = rng.standard_normal((3 * D,), dtype=np.float32) * 0.02
    wp = rng.standard_normal((D, D), dtype=np.float32) * 0.02
    bp = rng.standard_normal((D,), dtype=np.float32) * 0.02
    out = kernel(hs, wa, ba, wp, bp)
    print("out", out.shape, out.dtype, float(np.abs(out).max()))
